# revision 6
# baseline (speedup 1.0000x reference)
"""Trainium2 Bass kernel for the nn_MultiHeadAttention problem.

Data-parallel over batch: each of the 8 NeuronCores processes one batch
element independently (no collectives).

Per-core dataflow (L=1024, E=1024, H=16, D=64; q/k tiles of 128,
e-chunks of 128 = 2 heads):

  host:  QT/KT/VT = Q[b].T etc (so no on-chip input transposes),
         W2[c] = blockdiag(HeadLinear[2c], HeadLinear[2c+1]),
         masks as f32 [128, 8] in tile layout, O in bf16.
  proj:  qT2/kT2 [128,1024] = W2[c].T @ QT_chunk (f32r, both heads at once)
         v2 [128k,130] per k-tile = (VT_chunk.T @ W2b[c]) * kmask, plus a
         kmask "ones" column per head (bf16).
  scores: s[k,q] psum = kT_h_slice.T @ qT_h (f32r); P = exp(s/8) via one
         ACT op per [128,1024] tile, output bf16.  No max subtraction
         (|s|/8 <~ 13), no explicit -1e10 mask: masked keys have v-rows
         and ones-column zeroed, which reproduces masked_fill+softmax
         exactly (exp(-1e10) == 0 in f32).
  PV:    out[q,65] psum = sum_k P^T_slice.T @ v2_slice (bf16); col 64 is
         the softmax denominator.  One DVE divide per (q-tile, head)
         normalizes and writes bf16 C chunk.
  CT:    per-chunk PE transposes C[q,e_chunk] -> CT[e,q] (bf16).
  final: Y[q,:] psum = sum_c CT_slice.T @ O_chunk (bf16), multiplied by
         the query mask during psum->sbuf evacuation, DMA'd out.
"""

import os
import sys

import numpy as np

try:
    import concourse  # noqa: F401
except ImportError:  # pragma: no cover
    for _p in ("/opt/trn_rl_repo", os.path.expanduser("~/.axon_site/_ro/trn_rl_repo")):
        if os.path.isdir(_p) and _p not in sys.path:
            sys.path.insert(0, _p)

import ml_dtypes

import concourse.bass as bass
import concourse.tile as tile
from concourse import bacc, mybir
from concourse.masks import make_identity

B, L, E, H, D = 8, 1024, 1024, 16, 64
P = 128          # partitions
NT = L // P      # 8 q/k tiles
NCH = E // P     # 8 e-chunks (2 heads each)
F32 = mybir.dt.float32
F32R = mybir.dt.float32r
BF16 = mybir.dt.bfloat16


def build_bass():
    nc = bacc.Bacc(None, target_bir_lowering=False, debug=False)

    QT = nc.declare_dram_parameter("QT", [E, L], F32R, isOutput=False)
    KT = nc.declare_dram_parameter("KT", [E, L], F32R, isOutput=False)
    VT = nc.declare_dram_parameter("VT", [E, L], BF16, isOutput=False)
    W2 = nc.declare_dram_parameter("W2", [P, NCH, P], F32R, isOutput=False)
    W2b = nc.declare_dram_parameter("W2b", [P, NCH, P], BF16, isOutput=False)
    OB = nc.declare_dram_parameter("OB", [E, E], BF16, isOutput=False)
    KM = nc.declare_dram_parameter("KM", [P, NT], F32, isOutput=False)
    QM = nc.declare_dram_parameter("QM", [P, NT], F32, isOutput=False)
    Y = nc.declare_dram_parameter("Y", [L, E], F32, isOutput=True)

    with tile.TileContext(nc) as tc:
        with (
            tc.tile_pool(name="singles", bufs=1) as singles,
            tc.tile_pool(name="qkT", bufs=2) as qkT,
            tc.tile_pool(name="vaug", bufs=2) as vaug,
            tc.tile_pool(name="ppool", bufs=2) as ppool,
            tc.tile_pool(name="cchunk", bufs=2) as cchunk,
            tc.tile_pool(name="ystage", bufs=2) as ystage,
            tc.tile_pool(name="rpool", bufs=4) as rpool,
            tc.tile_pool(name="psbig", bufs=2, space="PSUM") as psbig,
            tc.tile_pool(name="pspv", bufs=2, space="PSUM") as pspv,
            tc.tile_pool(name="pssmall", bufs=2, space="PSUM") as pssmall,
        ):
            # --- persistent SBUF tensors -------------------------------
            qts = singles.tile([P, NCH, L], F32R)
            kts = singles.tile([P, NCH, L], F32R)
            vts = singles.tile([P, NCH, L], BF16)
            obs = singles.tile([P, NCH, E], BF16)
            w2s = singles.tile([P, NCH, P], F32R)
            w2bs = singles.tile([P, NCH, P], BF16)
            kms = singles.tile([P, NT], F32)
            qms = singles.tile([P, NT], F32)
            ident = singles.tile([P, P], BF16)
            ct = singles.tile([P, NCH, L], BF16)

            make_identity(nc, ident[:])

            # --- input DMAs (small/consts first, then per-chunk) -------
            nc.gpsimd.dma_start(out=w2s[:], in_=W2[:])
            nc.gpsimd.dma_start(out=w2bs[:], in_=W2b[:])
            nc.gpsimd.dma_start(out=kms[:], in_=KM[:])
            nc.gpsimd.dma_start(out=qms[:], in_=QM[:])
            for c in range(NCH):
                nc.gpsimd.dma_start(out=qts[:, c, :], in_=QT[c * P:(c + 1) * P, :])
                nc.gpsimd.dma_start(out=kts[:, c, :], in_=KT[c * P:(c + 1) * P, :])
                nc.gpsimd.dma_start(out=vts[:, c, :], in_=VT[c * P:(c + 1) * P, :])
            for c in range(NCH):
                nc.gpsimd.dma_start(out=obs[:, c, :], in_=OB[c * P:(c + 1) * P, :])

            # --- main loop over e-chunks (2 heads each) ----------------
            for c in range(NCH):
                # projections for both heads of this chunk
                qt2 = qkT.tile([P, L], F32R, tag="qt2")
                kt2 = qkT.tile([P, L], F32R, tag="kt2")
                for dst, src in ((qt2, qts), (kt2, kts)):
                    for hf in range(2):
                        ps = pssmall.tile([P, 512], F32, tag="small")
                        nc.tensor.matmul(
                            out=ps[:],
                            lhsT=w2s[:, c, :],
                            rhs=src[:, c, 512 * hf:512 * (hf + 1)],
                            start=True, stop=True,
                        )
                        nc.vector.tensor_copy(dst[:, 512 * hf:512 * (hf + 1)], ps[:])

                # v projection + key-mask + ones-column, both heads
                v2 = vaug.tile([P, NT, 130], BF16)
                for t in range(NT):
                    ps = pssmall.tile([P, P], F32, tag="small")
                    nc.tensor.matmul(
                        out=ps[:],
                        lhsT=vts[:, c, t * P:(t + 1) * P],
                        rhs=w2bs[:, c, :],
                        start=True, stop=True,
                    )
                    nc.vector.tensor_scalar(
                        out=v2[:, t, 0:64], in0=ps[:, 0:64],
                        scalar1=kms[:, t:t + 1], scalar2=None,
                        op0=mybir.AluOpType.mult,
                    )
                    nc.vector.tensor_scalar(
                        out=v2[:, t, 65:129], in0=ps[:, 64:128],
                        scalar1=kms[:, t:t + 1], scalar2=None,
                        op0=mybir.AluOpType.mult,
                    )
                # denominator "ones" columns = key mask itself
                nc.vector.tensor_copy(v2[:, :, 64], kms[:, :])
                nc.vector.tensor_copy(v2[:, :, 129], kms[:, :])

                cc = cchunk.tile([P, NT, P], BF16)
                for hf in range(2):
                    hq = qt2[64 * hf:64 * hf + 64, :]
                    hk = kt2[64 * hf:64 * hf + 64, :]
                    # scores (transposed, [k, q]) + exp -> P (bf16)
                    pt = ppool.tile([P, NT, L], BF16)
                    for t in range(NT):
                        sps = psbig.tile([P, L], F32, tag="big")
                        for qh in range(2):
                            nc.tensor.matmul(
                                out=sps[:, 512 * qh:512 * (qh + 1)],
                                lhsT=hk[:, t * P:(t + 1) * P],
                                rhs=hq[:, 512 * qh:512 * (qh + 1)],
                                start=True, stop=True,
                            )
                        nc.scalar.activation(
                            out=pt[:, t, :], in_=sps[:],
                            func=mybir.ActivationFunctionType.Exp,
                            scale=0.125,
                        )
                    # P^T @ v_aug, then normalize by the ones-column
                    for t in range(NT):
                        ops = pspv.tile([P, 65], F32)
                        for kt in range(NT):
                            nc.tensor.matmul(
                                out=ops[:],
                                lhsT=pt[:, kt, t * P:(t + 1) * P],
                                rhs=v2[:, kt, 65 * hf:65 * hf + 65],
                                start=(kt == 0), stop=(kt == NT - 1),
                            )
                        rec = rpool.tile([P, 1], F32)
                        nc.vector.reciprocal(out=rec[:], in_=ops[:, 64:65])
                        nc.vector.tensor_scalar(
                            out=cc[:, t, 64 * hf:64 * hf + 64],
                            in0=ops[:, 0:64],
                            scalar1=rec[:], scalar2=None,
                            op0=mybir.AluOpType.mult,
                        )

                # transpose C chunk [q, e] -> CT [e, q]
                for t in range(NT):
                    tp = pssmall.tile([P, P], BF16, tag="small")
                    nc.tensor.transpose(out=tp[:], in_=cc[:, t, :], identity=ident[:])
                    nc.vector.tensor_copy(ct[:, c, t * P:(t + 1) * P], tp[:])

            # --- output projection ------------------------------------
            for t in range(NT):
                yps = psbig.tile([P, E], F32, tag="big")
                for c in range(NCH):
                    for eh in range(2):
                        nc.tensor.matmul(
                            out=yps[:, 512 * eh:512 * (eh + 1)],
                            lhsT=ct[:, c, t * P:(t + 1) * P],
                            rhs=obs[:, c, 512 * eh:512 * (eh + 1)],
                            start=(c == 0), stop=(c == NCH - 1),
                        )
                ys = ystage.tile([P, E], F32)
                nc.vector.tensor_scalar(
                    out=ys[:], in0=yps[:],
                    scalar1=qms[:, t:t + 1], scalar2=None,
                    op0=mybir.AluOpType.mult,
                )
                nc.gpsimd.dma_start(out=Y[t * P:(t + 1) * P, :], in_=ys[:])

    nc.compile()
    return nc


def make_core_inputs(Q, K, V, HeadLinear, OutputLiner, QMask, KMask):
    """Host-side sharding/layout prep. Returns list of per-core in_maps."""
    bf16 = ml_dtypes.bfloat16
    w2 = np.zeros((P, NCH, P), dtype=np.float32)
    hl = np.asarray(HeadLinear, dtype=np.float32)
    for c in range(NCH):
        w2[0:64, c, 0:64] = hl[2 * c]
        w2[64:128, c, 64:128] = hl[2 * c + 1]
    w2b = w2.astype(bf16)
    ob = np.asarray(OutputLiner, dtype=np.float32).astype(bf16)

    in_maps = []
    for b in range(B):
        qt = np.ascontiguousarray(np.asarray(Q[b], dtype=np.float32).T)
        kt = np.ascontiguousarray(np.asarray(K[b], dtype=np.float32).T)
        vt = np.ascontiguousarray(np.asarray(V[b], dtype=np.float32).T).astype(bf16)
        km = np.ascontiguousarray(
            np.asarray(KMask[b]).astype(np.float32).reshape(NT, P).T)
        qm = np.ascontiguousarray(
            np.asarray(QMask[b]).astype(np.float32).reshape(NT, P).T)
        in_maps.append({
            "QT": qt, "KT": kt, "VT": vt,
            "W2": w2, "W2b": w2b, "OB": ob,
            "KM": km, "QM": qm,
        })
    return in_maps


_NC_CACHE = None


def _get_nc():
    global _NC_CACHE
    if _NC_CACHE is None:
        _NC_CACHE = build_bass()
    return _NC_CACHE


def kernel(Q, K, V, HeadLinear, OutputLiner, QMask, KMask):
    from concourse.bass_utils import run_bass_kernel_spmd

    nc = _get_nc()
    in_maps = make_core_inputs(Q, K, V, HeadLinear, OutputLiner, QMask, KMask)
    res = run_bass_kernel_spmd(nc, in_maps, list(range(B)))
    out = np.stack([np.asarray(res.results[i]["Y"]) for i in range(B)])
    return out.astype(np.float32)
